# revision 39
# baseline (speedup 1.0000x reference)
"""Block-sparse local+strided attention (LocalStridedBlockSparseAttn) on 8 trn2 cores.

Problem: q,k,v [4096, 16, 64] f32, single prefill sequence. Per-head block mask
(64x64 token blocks): j <= i and (i - j < 8  or  (j + h + 1) % 8 == 0).

Sharding: core c owns heads {c, c+8} - both have the same strided residue
s = (7 - c) % 8, so one SPMD program serves all 8 cores with per-core data.

v4 (instruction-minimal dataflow; v3 was PE-bound on per-instruction fixed
costs and startup DMA serialization):
  - local part per (chunk, head): one [128, w] matmul per k-block PAIR over
    its contiguous valid q-window (w up to 512), masked post-exp with small
    constant masks, plus one 64x64 tail piece; small pieces packed into
    shared PSUM banks so ONE activation serves several matmuls.
  - strided validity boundary folded into the CONTRACTION: ks carries 7
    indicator partitions and the strided q copy carries -1e9 rows on the
    boundary-chunk prefix columns (zero per-piece instructions).
  - output stays TRANSPOSED with the rowsums row: the [65, 512] PSUM tile
    [O^T; rowsums] is DMA'd straight to DRAM; the host does the divide and
    the final transpose. No PE transposes, no reciprocal/normalize/copy
    instructions on device.
  - big inputs split in halves with chunk-0-critical slices DMA'd first so
    compute starts before the tail of the input load.
All matmul operands bf16; exp outputs bf16 (PSUM accumulates fp32).
"""

import numpy as np

N_HEADS = 16
HEAD = 64
SEQ = 4096
BS = 64
NB = 64          # 64 token-blocks
LOCAL = 8
VERT = 8
SM_SCALE = 1.0 / 8.0
NCORES = 8
CHUNK = 512      # q tokens per chunk (8 blocks)
NCH = SEQ // CHUNK
NSB = 8          # packed strided block slots (7 real, slot 7 zero pad)
KSP = 64 + 7     # ks/qs partitions: 64 head dims + 7 boundary indicator rows
HSEQ = SEQ // 2

_cache = {}


def _legalize_waits(nc, max_waits=1):
    """This walrus build rejects instructions carrying more than one sync-wait
    condition ("Too many sync wait commands"); hoist extras into same-engine
    NoOps placed immediately before the instruction."""
    import concourse.mybir as mybir

    nid = 0
    for bb in nc.main_func.blocks:
        new = []
        for ins in bb.instructions:
            si = ins.sync_info
            if si is not None and si.on_wait and len(si.on_wait) > max_waits:
                waits = list(si.on_wait)
                while len(waits) > max_waits:
                    chunk, waits = waits[:max_waits], waits[max_waits:]
                    nid += 1
                    nop = mybir.InstNoOp(name=f"{ins.name}-wsplit{nid}")
                    nop.engine = ins.engine
                    nop.sync_info = mybir.SyncInfo(on_wait=chunk, on_update=[])
                    new.append(nop)
                ins.sync_info = mybir.SyncInfo(on_wait=waits,
                                               on_update=list(si.on_update))
            new.append(ins)
        bb.instructions[:] = new
    return nc


def _build_program(chunks=None, heads=(0, 1)):
    from contextlib import ExitStack

    import concourse.bass as bass
    import concourse.mybir as mybir
    from concourse import tile

    f32 = mybir.dt.float32
    bf16 = mybir.dt.bfloat16
    Exp = mybir.ActivationFunctionType.Exp

    nc = bass.Bass()
    qT_d = nc.dram_tensor("qT", [128, SEQ], bf16, kind="ExternalInput")
    kT_d = nc.dram_tensor("kT", [128, SEQ], bf16, kind="ExternalInput")
    ksT_d = nc.dram_tensor("ksT", [128, NSB * BS], bf16, kind="ExternalInput")
    vaug_d = nc.dram_tensor("vaug", [128, 32 * 130], bf16, kind="ExternalInput")
    vsaug_d = nc.dram_tensor("vsaug", [128, 4 * 130], bf16, kind="ExternalInput")
    # per-core strided boundary masks (all-ones prefix zeroed up to s*64):
    # mbH for a boundary block in the pair's high half, mbL for a lone one.
    mbH_d = nc.dram_tensor("mbH", [128, CHUNK], bf16, kind="ExternalInput")
    mbL_d = nc.dram_tensor("mbL", [64, CHUNK], bf16, kind="ExternalInput")
    # transposed output with rowsums: rows h*65..h*65+63 = O^T, row h*65+64 =
    # softmax denominators; host divides + transposes back.
    outT_d = nc.dram_tensor("outT", [130, SEQ], f32, kind="ExternalOutput")

    # Device-constant tiles (same on every core).
    import ml_dtypes
    bf = ml_dtypes.bfloat16
    n = np.arange(64)
    tri = (n[None, :] >= n[:, None]).astype(np.float32)
    m01_np = np.zeros((128, 128), np.float32)
    m01_np[:64, :64] = tri          # q-block 2p vs k-block 2p
    m01_np[:64, 64:] = 1.0          # q-block 2p+1 vs k-block 2p
    m01_np[64:, 64:] = tri          # q-block 2p+1 vs k-block 2p+1
    mB_np = np.zeros((128, 64), np.float32)
    mB_np[64:] = 1.0                # q-block 2p+8: only k-block 2p+1 valid
    m01_d = nc.inline_tensor(m01_np.astype(bf), "m01_c")
    mB_d = nc.inline_tensor(mB_np.astype(bf), "mB_c")

    with tile.TileContext(nc) as tc, ExitStack() as ctx:
        const = ctx.enter_context(tc.tile_pool(name="const", bufs=1))
        m01 = const.tile([128, 128], bf16, tag="m01")
        mB = const.tile([128, 64], bf16, tag="mB")

        big = ctx.enter_context(tc.tile_pool(name="big", bufs=1))
        # quarters (1024 cols) so chunk 0 starts after ~1/4 of the load;
        # issue on BOTH hwdge queues (sync + scalar) in need-order.
        # k/q column slabs; the first two are small so chunk 0 starts ASAP
        SLAB = [0, 512, 1024, 2048, 3072, SEQ]
        kTq = [big.tile([128, SLAB[i + 1] - SLAB[i]], bf16,
                        tag=f"kT{i}", name=f"kT{i}") for i in range(5)]
        qTq = [big.tile([128, SLAB[i + 1] - SLAB[i]], bf16,
                        tag=f"qT{i}", name=f"qT{i}") for i in range(5)]
        ksT = big.tile([128, NSB * BS], bf16, tag="ksT")
        vaugh = [big.tile([128, 16 * 130], bf16, tag=f"vaug{i}", name=f"vaug{i}")
                 for i in range(2)]
        vsaug = big.tile([128, 4 * 130], bf16, tag="vsaug")
        mbH = big.tile([128, CHUNK], bf16, tag="mbH")
        mbL = big.tile([64, CHUNK], bf16, tag="mbL")
        # scalar queue: small early tensors + chunk-0/1 V layouts (the scalar
        # sequencer must drain its DMA issues before it can run the first exp)
        nc.scalar.dma_start(ksT[:], ksT_d[:])
        nc.scalar.dma_start(vsaug[:], vsaug_d[:])
        nc.scalar.dma_start(mbH[:], mbH_d[:])
        nc.scalar.dma_start(mbL[:], mbL_d[:])
        nc.scalar.dma_start(vaugh[0][:], vaug_d[:, 0:16 * 130])
        # sync queue: everything else in need-order; the chunk-0 k/q slabs
        # go absolutely first (they gate the first matmul), constants after
        for i in range(5):
            nc.sync.dma_start(kTq[i][:], kT_d[:, SLAB[i]:SLAB[i + 1]])
            nc.sync.dma_start(qTq[i][:], qT_d[:, SLAB[i]:SLAB[i + 1]])
            if i == 0:
                nc.sync.dma_start(m01[:], m01_d[:])
                nc.sync.dma_start(mB[:], mB_d[:])
            if i == 3:
                nc.sync.dma_start(vaugh[1][:], vaug_d[:, 16 * 130:32 * 130])

        import bisect

        def kT_ap(hq, col, w):
            i = bisect.bisect_right(SLAB, col) - 1
            return kTq[i][hq, col - SLAB[i]:col - SLAB[i] + w]

        def qT_ap(hq, col, w):
            i = bisect.bisect_right(SLAB, col) - 1
            return qTq[i][hq, col - SLAB[i]:col - SLAB[i] + w]

        def vaug_ap(np_, p, off, w):
            i, base = (0, 0) if p < 16 else (1, 16)
            return vaugh[i][:np_, (p - base) * 130 + off:(p - base) * 130 + off + w]

        def vaug_ap2(p, off):
            # high half of pair p = tokens of odd block 2p+1 (tail PV lhsT)
            i, base = (0, 0) if p < 16 else (1, 16)
            return vaugh[i][64:128, (p - base) * 130 + off:(p - base) * 130 + off + 65]

        # ---- chunked attention ----
        # psD tiles span TWO PSUM banks ([128, 1024]) so one exp instruction
        # serves two 512-col score groups.
        expp = ctx.enter_context(tc.tile_pool(name="expp", bufs=20))
        psD = ctx.enter_context(tc.tile_pool(name="psD", bufs=3, space="PSUM"))
        psOT = ctx.enter_context(tc.tile_pool(name="psOT", bufs=2, space="PSUM"))
        sot = ctx.enter_context(tc.tile_pool(name="sot", bufs=2))

        def emit_phase2_head(pieces, c, h):
            # PV accumulation + output for one head. First piece must cover
            # the full 512 cols (start=True replaces a memset).
            ot = psOT.tile([65, CHUNK], f32, tag="psOT", name=f"ot{c}_{h}")
            ffull = next(i for i, pc in enumerate(pieces) if pc[3] == CHUNK)
            pieces[0], pieces[ffull] = pieces[ffull], pieces[0]
            for pi, (et, vl, col, w) in enumerate(pieces):
                nc.tensor.matmul(ot[:, col:col + w], vl, et,
                                 start=(pi == 0),
                                 stop=(pi == len(pieces) - 1),
                                 skip_group_check=True)
            # [O^T; rowsums] to DRAM via SBUF staging; host normalizes.
            so = sot.tile([65, CHUNK], f32, tag="sot")
            nc.vector.tensor_copy(so[:], ot[:])
            nc.sync.dma_start(
                outT_d[h * 65:(h + 1) * 65, c * CHUNK:(c + 1) * CHUNK],
                so[:])

        prev = None
        for c in (range(NCH) if chunks is None else chunks):
            # phase 1: scores + exp (+ masks) for BOTH heads, so the PE can
            # stream head 1's scores while head 0's exps drain on ACT.
            # The PREVIOUS chunk's PVs are emitted after this chunk's scores
            # (software pipeline): PVs then never wait on exps, and ACT chews
            # this chunk's exps while the PE runs the previous chunk's PVs.
            pieces_h = {}
            for h in heads:
                hq = slice(h * 64, (h + 1) * 64)
                hv = h * 65
                pieces = pieces_h[h] = []  # (et ap, vl ap, ot_col, w)

                # build 512-col sub-bank groups first, then pair them into
                # two-bank [128, 1024] PSUM tiles with ONE exp each.
                # strided pieces (full-width; the boundary block's invalid
                # prefix columns are zeroed post-exp with a per-core mask):
                subbanks = []  # (used, [(p, wlo, whi, npart, kind, off)])
                for pr in range((c + 1) // 2):
                    npart = 128 if 2 * pr + 1 < c else 64
                    subbanks.append((CHUNK, [(pr, 8 * c, 8 * c + 7, npart, 3, 0)]))
                # local: one piece per k-pair window (+ tail)
                locs = []  # (p, wlo, whi, npart, kind)
                for p in range(max(0, 4 * c - 4), min(31, 4 * c + 3) + 1):
                    if p == 4 * c - 4:
                        locs.append((p, 8 * c, 8 * c, 64, 2))     # tail
                        continue
                    wlo = max(8 * c, 2 * p)
                    whi = min(8 * c + 7, 2 * p + 8, 63)
                    if wlo > whi:
                        continue
                    locs.append((p, wlo, whi, 128, 0 if p >= 4 * c else 1))
                banks = []  # [used, [(p, wlo, whi, npart, kind, off), ...]]
                for ent in sorted(locs, key=lambda e: -(e[2] - e[1] + 1)):
                    w = (ent[2] - ent[1] + 1) * 64
                    for bk in banks:
                        if bk[0] + w <= CHUNK:
                            bk[1].append(ent + (bk[0],))
                            bk[0] += w
                            break
                    else:
                        banks.append([w, [ent + (0,)]])
                subbanks.extend((u, s) for u, s in banks)

                for g in range(0, len(subbanks), 2):
                    pair = subbanks[g:g + 2]
                    ps = psD.tile([128, 2 * CHUNK], f32, tag="psD")
                    et = expp.tile([128, 2 * CHUNK], bf16, tag="exp")
                    for half, (used, subs) in enumerate(pair):
                        hb = half * CHUNK
                        for p, wlo, whi, npart, kind, off in subs:
                            w = (whi - wlo + 1) * 64
                            if kind == 3:
                                nc.tensor.matmul(
                                    ps[:npart, hb:hb + CHUNK],
                                    ksT[hq, p * 128:p * 128 + npart],
                                    qT_ap(hq, 8 * c * 64, CHUNK),
                                    start=True, stop=True,
                                    skip_group_check=True)
                            elif kind == 2:
                                # tail targets partitions 64-127 so its PV
                                # can use vaug's high half directly
                                nc.tensor.matmul(
                                    ps[64:128, hb + off:hb + off + w],
                                    kT_ap(hq, (2 * p + 1) * 64, 64),
                                    qT_ap(hq, wlo * 64, w),
                                    start=True, stop=True,
                                    skip_group_check=True)
                            else:
                                nc.tensor.matmul(
                                    ps[:npart, hb + off:hb + off + w],
                                    kT_ap(hq, 2 * p * 64, 128),
                                    qT_ap(hq, wlo * 64, w),
                                    start=True, stop=True,
                                    skip_group_check=True)
                    width = (CHUNK + pair[1][0]) if len(pair) == 2 else pair[0][0]
                    nc.scalar.activation(et[:, :width], ps[:, :width], Exp,
                                         scale=SM_SCALE)
                    for half, (used, subs) in enumerate(pair):
                        hb = half * CHUNK
                        for p, wlo, whi, npart, kind, off in subs:
                            w = (whi - wlo + 1) * 64
                            if kind == 0:
                                nc.vector.tensor_mul(
                                    et[:, hb + off:hb + off + 128],
                                    et[:, hb + off:hb + off + 128], m01[:])
                            elif kind == 1:
                                nc.vector.tensor_mul(
                                    et[:, hb + off + w - 64:hb + off + w],
                                    et[:, hb + off + w - 64:hb + off + w],
                                    mB[:])
                            elif kind == 3 and p == (c - 1) // 2:
                                # strided boundary block: zero the invalid
                                # prefix columns (per-core mask data)
                                if npart == 64:
                                    nc.vector.tensor_mul(
                                        et[:64, hb:hb + CHUNK],
                                        et[:64, hb:hb + CHUNK], mbL[:])
                                else:
                                    nc.vector.tensor_mul(
                                        et[:, hb:hb + CHUNK],
                                        et[:, hb:hb + CHUNK], mbH[:])
                            if kind == 3:
                                vl = vsaug[:npart, p * 130 + hv:p * 130 + hv + 65]
                                ea = et[:npart, hb:hb + CHUNK]
                            elif kind == 2:
                                vl = vaug_ap2(p, hv)
                                ea = et[64:128, hb + off:hb + off + w]
                            else:
                                vl = vaug_ap(npart, p, hv, 65)
                                ea = et[:npart, hb + off:hb + off + w]
                            pieces.append((ea, vl, (wlo - 8 * c) * 64, w))

                # previous chunk's PV for this head goes right here, between
                # the two heads' score phases: the PE runs ready PV work
                # while this head's exps drain and psD tiles recycle.
                if prev is not None:
                    emit_phase2_head(prev[0][h], prev[1], h)
            prev = (pieces_h, c)
        for h in heads:
            emit_phase2_head(prev[0][h], prev[1], h)

    return nc


def _in_maps(q, k, v):
    import ml_dtypes
    bf = ml_dtypes.bfloat16
    maps = []
    for c in range(NCORES):
        heads = [c, c + 8]
        s = (7 - c) % 8
        qT = np.ascontiguousarray(q[:, heads, :].reshape(SEQ, 128).T).astype(bf)
        kT = np.ascontiguousarray(k[:, heads, :].reshape(SEQ, 128).T).astype(bf)
        # packed strided k blocks (7 real + zero pad), transposed, and the
        # boundary masks (zero the first s*64 columns of the boundary chunk)
        ksb = np.zeros((NSB * BS, 128), np.float32)
        vsb = np.zeros((NSB, BS, 128), np.float32)
        for b in range(7):
            j = s + 8 * b
            ksb[b * BS:(b + 1) * BS] = k[j * BS:(j + 1) * BS, heads, :].reshape(BS, 128)
            vsb[b] = v[j * BS:(j + 1) * BS, heads, :].reshape(BS, 128)
        ksT = np.ascontiguousarray(ksb.T).astype(bf)
        mbH = np.ones((128, CHUNK), np.float32)
        mbH[64:, :s * 64] = 0.0
        mbL = np.ones((64, CHUNK), np.float32)
        mbL[:, :s * 64] = 0.0
        # vaug [128, 32*130]: pair a, token p -> [V_h0 | 1 | V_h1 | 1]
        vv = v[:, heads, :].reshape(32, 128, 128)   # [a, p, hd]
        vaug = np.ones((128, 32, 130), np.float32)
        vaug[:, :, 0:64] = vv.transpose(1, 0, 2)[:, :, 0:64]
        vaug[:, :, 65:129] = vv.transpose(1, 0, 2)[:, :, 64:128]
        # vsaug [128, 4*130]: pair pr: partitions 0-63 = block 2pr, 64-127 =
        # block 2pr+1
        vsp = vsb.reshape(4, 2, BS, 128).transpose(1, 2, 0, 3).reshape(128, 4, 128)
        vsaug = np.ones((128, 4, 130), np.float32)
        vsaug[:, :, 0:64] = vsp[:, :, 0:64]
        vsaug[:, :, 65:129] = vsp[:, :, 64:128]
        maps.append({"qT": qT, "kT": kT, "ksT": ksT,
                     "vaug": vaug.reshape(128, 32 * 130).astype(bf),
                     "vsaug": vsaug.reshape(128, 4 * 130).astype(bf),
                     "mbH": mbH.astype(bf), "mbL": mbL.astype(bf)})
    return maps


def kernel(q, k, v, cu_seqlens_k=None, **_):
    from concourse.bass_utils import run_bass_kernel_spmd

    q = np.asarray(q, np.float32)
    k = np.asarray(k, np.float32)
    v = np.asarray(v, np.float32)
    if "nc" not in _cache:
        _cache["nc"] = _legalize_waits(_build_program())
    res = run_bass_kernel_spmd(_cache["nc"], _in_maps(q, k, v),
                               list(range(NCORES))).results
    out = np.empty((SEQ, N_HEADS, HEAD), np.float32)
    for c in range(NCORES):
        o = res[c]["outT"]                      # [130, SEQ]
        for hh, head in ((0, c), (1, c + 8)):
            num = o[hh * 65:hh * 65 + 64, :]    # [64, SEQ]
            den = o[hh * 65 + 64, :]            # [SEQ]
            out[:, head, :] = (num / den).T
    return out


# revision 42
# speedup vs baseline: 1.0898x; 1.0898x over previous
"""Block-sparse local+strided attention (LocalStridedBlockSparseAttn) on 8 trn2 cores.

Problem: q,k,v [4096, 16, 64] f32, single prefill sequence. Per-head block mask
(64x64 token blocks): j <= i and (i - j < 8  or  (j + h + 1) % 8 == 0).

Sharding: core c owns heads {c, c+8} - both have the same strided residue
s = (7 - c) % 8, so one SPMD program serves all 8 cores with per-core data.

v4 (instruction-minimal dataflow; v3 was PE-bound on per-instruction fixed
costs and startup DMA serialization):
  - local part per (chunk, head): one [128, w] matmul per k-block PAIR over
    its contiguous valid q-window (w up to 512), masked post-exp with small
    constant masks, plus one 64x64 tail piece; small pieces packed into
    shared PSUM banks so ONE activation serves several matmuls.
  - strided validity boundary folded into the CONTRACTION: ks carries 7
    indicator partitions and the strided q copy carries -1e9 rows on the
    boundary-chunk prefix columns (zero per-piece instructions).
  - output stays TRANSPOSED with the rowsums row: the [65, 512] PSUM tile
    [O^T; rowsums] is DMA'd straight to DRAM; the host does the divide and
    the final transpose. No PE transposes, no reciprocal/normalize/copy
    instructions on device.
  - big inputs split in halves with chunk-0-critical slices DMA'd first so
    compute starts before the tail of the input load.
All matmul operands bf16; exp outputs bf16 (PSUM accumulates fp32).
"""

import numpy as np

N_HEADS = 16
HEAD = 64
SEQ = 4096
BS = 64
NB = 64          # 64 token-blocks
LOCAL = 8
VERT = 8
SM_SCALE = 1.0 / 8.0
NCORES = 8
CHUNK = 512      # q tokens per chunk (8 blocks)
NCH = SEQ // CHUNK
NSB = 8          # packed strided block slots (7 real, slot 7 zero pad)
KSP = 64 + 7     # ks/qs partitions: 64 head dims + 7 boundary indicator rows
HSEQ = SEQ // 2

_cache = {}


def _legalize_waits(nc, max_waits=1):
    """This walrus build rejects instructions carrying more than one sync-wait
    condition ("Too many sync wait commands"); hoist extras into same-engine
    NoOps placed immediately before the instruction."""
    import concourse.mybir as mybir

    nid = 0
    for bb in nc.main_func.blocks:
        new = []
        for ins in bb.instructions:
            si = ins.sync_info
            if si is not None and si.on_wait and len(si.on_wait) > max_waits:
                waits = list(si.on_wait)
                while len(waits) > max_waits:
                    chunk, waits = waits[:max_waits], waits[max_waits:]
                    nid += 1
                    nop = mybir.InstNoOp(name=f"{ins.name}-wsplit{nid}")
                    nop.engine = ins.engine
                    nop.sync_info = mybir.SyncInfo(on_wait=chunk, on_update=[])
                    new.append(nop)
                ins.sync_info = mybir.SyncInfo(on_wait=waits,
                                               on_update=list(si.on_update))
            new.append(ins)
        bb.instructions[:] = new
    return nc


def _build_program(chunks=None, heads=(0, 1)):
    from contextlib import ExitStack

    import concourse.bass as bass
    import concourse.mybir as mybir
    from concourse import tile

    f32 = mybir.dt.float32
    bf16 = mybir.dt.bfloat16
    Exp = mybir.ActivationFunctionType.Exp

    nc = bass.Bass()
    qT_d = nc.dram_tensor("qT", [128, SEQ], bf16, kind="ExternalInput")
    kT_d = nc.dram_tensor("kT", [128, SEQ], bf16, kind="ExternalInput")
    ksT_d = nc.dram_tensor("ksT", [128, NSB * BS], bf16, kind="ExternalInput")
    vaug_d = nc.dram_tensor("vaug", [128, 32 * 130], bf16, kind="ExternalInput")
    vsaug_d = nc.dram_tensor("vsaug", [128, 4 * 130], bf16, kind="ExternalInput")
    # per-core strided boundary masks (all-ones prefix zeroed up to s*64):
    # mbH for a boundary block in the pair's high half, mbL for a lone one.
    mbH_d = nc.dram_tensor("mbH", [128, CHUNK], bf16, kind="ExternalInput")
    mbL_d = nc.dram_tensor("mbL", [64, CHUNK], bf16, kind="ExternalInput")
    # transposed output with rowsums: rows h*65..h*65+63 = O^T, row h*65+64 =
    # softmax denominators; host divides + transposes back.
    outT_d = nc.dram_tensor("outT", [130, SEQ], f32, kind="ExternalOutput")

    # Device-constant tiles (same on every core).
    import ml_dtypes
    bf = ml_dtypes.bfloat16
    n = np.arange(64)
    tri = (n[None, :] >= n[:, None]).astype(np.float32)
    m01_np = np.zeros((128, 128), np.float32)
    m01_np[:64, :64] = tri          # q-block 2p vs k-block 2p
    m01_np[:64, 64:] = 1.0          # q-block 2p+1 vs k-block 2p
    m01_np[64:, 64:] = tri          # q-block 2p+1 vs k-block 2p+1
    mB_np = np.zeros((128, 64), np.float32)
    mB_np[64:] = 1.0                # q-block 2p+8: only k-block 2p+1 valid
    m01_d = nc.inline_tensor(m01_np.astype(bf), "m01_c")
    mB_d = nc.inline_tensor(mB_np.astype(bf), "mB_c")

    with tile.TileContext(nc) as tc, ExitStack() as ctx:
        const = ctx.enter_context(tc.tile_pool(name="const", bufs=1))
        m01 = const.tile([128, 128], bf16, tag="m01")
        mB = const.tile([128, 64], bf16, tag="mB")

        big = ctx.enter_context(tc.tile_pool(name="big", bufs=1))
        # quarters (1024 cols) so chunk 0 starts after ~1/4 of the load;
        # issue on BOTH hwdge queues (sync + scalar) in need-order.
        # k/q column slabs; the first two are small so chunk 0 starts ASAP
        SLAB = [0, 512, 1024, 2048, 3072, SEQ]
        kTq = [big.tile([128, SLAB[i + 1] - SLAB[i]], bf16,
                        tag=f"kT{i}", name=f"kT{i}") for i in range(5)]
        qTq = [big.tile([128, SLAB[i + 1] - SLAB[i]], bf16,
                        tag=f"qT{i}", name=f"qT{i}") for i in range(5)]
        ksT = big.tile([128, NSB * BS], bf16, tag="ksT")
        vaugh = [big.tile([128, 16 * 130], bf16, tag=f"vaug{i}", name=f"vaug{i}")
                 for i in range(2)]
        vsaug = big.tile([128, 4 * 130], bf16, tag="vsaug")
        mbH = big.tile([128, CHUNK], bf16, tag="mbH")
        mbL = big.tile([64, CHUNK], bf16, tag="mbL")
        # scalar queue: small early tensors + chunk-0/1 V layouts (the scalar
        # sequencer must drain its DMA issues before it can run the first exp)
        nc.scalar.dma_start(ksT[:], ksT_d[:])
        nc.scalar.dma_start(vsaug[:], vsaug_d[:])
        nc.scalar.dma_start(mbH[:], mbH_d[:])
        nc.scalar.dma_start(mbL[:], mbL_d[:])
        nc.scalar.dma_start(vaugh[0][:], vaug_d[:, 0:16 * 130])
        # sync queue: everything else in need-order; the chunk-0 k/q slabs
        # go absolutely first (they gate the first matmul), constants after
        for i in range(5):
            nc.sync.dma_start(kTq[i][:], kT_d[:, SLAB[i]:SLAB[i + 1]])
            nc.sync.dma_start(qTq[i][:], qT_d[:, SLAB[i]:SLAB[i + 1]])
            if i == 0:
                nc.sync.dma_start(m01[:], m01_d[:])
                nc.sync.dma_start(mB[:], mB_d[:])
            if i == 3:
                nc.sync.dma_start(vaugh[1][:], vaug_d[:, 16 * 130:32 * 130])

        import bisect

        def kT_ap(hq, col, w):
            i = bisect.bisect_right(SLAB, col) - 1
            return kTq[i][hq, col - SLAB[i]:col - SLAB[i] + w]

        def qT_ap(hq, col, w):
            i = bisect.bisect_right(SLAB, col) - 1
            return qTq[i][hq, col - SLAB[i]:col - SLAB[i] + w]

        def vaug_ap(np_, p, off, w):
            i, base = (0, 0) if p < 16 else (1, 16)
            return vaugh[i][:np_, (p - base) * 130 + off:(p - base) * 130 + off + w]

        def vaug_ap2(p, off):
            # high half of pair p = tokens of odd block 2p+1 (tail PV lhsT)
            i, base = (0, 0) if p < 16 else (1, 16)
            return vaugh[i][64:128, (p - base) * 130 + off:(p - base) * 130 + off + 65]

        # ---- chunked attention ----
        # psD tiles span TWO PSUM banks ([128, 1024]) so one exp instruction
        # serves two 512-col score groups.
        expp = ctx.enter_context(tc.tile_pool(name="expp", bufs=20))
        psD = ctx.enter_context(tc.tile_pool(name="psD", bufs=3, space="PSUM"))
        psOT = ctx.enter_context(tc.tile_pool(name="psOT", bufs=2, space="PSUM"))
        sot = ctx.enter_context(tc.tile_pool(name="sot", bufs=2))

        def emit_phase2(pieces_h, c):
            # PV accumulation + output. The two heads' accumulation chains
            # are interleaved so consecutive PE instructions target different
            # PSUM banks (hides the SBUF access latency). First piece per
            # head must cover the full 512 cols (start=True replaces memset).
            ots = {}
            for h in heads:
                pieces = pieces_h[h]
                ots[h] = psOT.tile([65, CHUNK], f32, tag="psOT",
                                   name=f"ot{c}_{h}")
                ffull = next(i for i, pc in enumerate(pieces) if pc[3] == CHUNK)
                pieces[0], pieces[ffull] = pieces[ffull], pieces[0]
            nmax = max(len(pieces_h[h]) for h in heads)
            for pi in range(nmax):
                for h in heads:
                    pieces = pieces_h[h]
                    if pi >= len(pieces):
                        continue
                    et, vl, col, w = pieces[pi]
                    nc.tensor.matmul(ots[h][:, col:col + w], vl, et,
                                     start=(pi == 0),
                                     stop=(pi == len(pieces) - 1),
                                     skip_group_check=True)
            for h in heads:
                # [O^T; rowsums] to DRAM via SBUF staging; host normalizes.
                so = sot.tile([65, CHUNK], f32, tag="sot")
                nc.vector.tensor_copy(so[:], ots[h][:])
                nc.sync.dma_start(
                    outT_d[h * 65:(h + 1) * 65, c * CHUNK:(c + 1) * CHUNK],
                    so[:])

        prev = None
        for c in (range(NCH) if chunks is None else chunks):
            # phase 1: scores + exp (+ masks) for BOTH heads, so the PE can
            # stream head 1's scores while head 0's exps drain on ACT.
            # The PREVIOUS chunk's PVs are emitted after this chunk's scores
            # (software pipeline): PVs then never wait on exps, and ACT chews
            # this chunk's exps while the PE runs the previous chunk's PVs.
            pieces_h = {}
            for h in heads:
                hq = slice(h * 64, (h + 1) * 64)
                hv = h * 65
                pieces = pieces_h[h] = []  # (et ap, vl ap, ot_col, w)

                # build 512-col sub-bank groups first, then pair them into
                # two-bank [128, 1024] PSUM tiles with ONE exp each.
                # strided pieces (full-width; the boundary block's invalid
                # prefix columns are zeroed post-exp with a per-core mask):
                subbanks = []  # (used, [(p, wlo, whi, npart, kind, off)])
                for pr in range((c + 1) // 2):
                    npart = 128 if 2 * pr + 1 < c else 64
                    subbanks.append((CHUNK, [(pr, 8 * c, 8 * c + 7, npart, 3, 0)]))
                # local: one piece per k-pair window (+ tail)
                locs = []  # (p, wlo, whi, npart, kind)
                for p in range(max(0, 4 * c - 4), min(31, 4 * c + 3) + 1):
                    if p == 4 * c - 4:
                        locs.append((p, 8 * c, 8 * c, 64, 2))     # tail
                        continue
                    wlo = max(8 * c, 2 * p)
                    whi = min(8 * c + 7, 2 * p + 8, 63)
                    if wlo > whi:
                        continue
                    locs.append((p, wlo, whi, 128, 0 if p >= 4 * c else 1))
                banks = []  # [used, [(p, wlo, whi, npart, kind, off), ...]]
                for ent in sorted(locs, key=lambda e: -(e[2] - e[1] + 1)):
                    w = (ent[2] - ent[1] + 1) * 64
                    for bk in banks:
                        if bk[0] + w <= CHUNK:
                            bk[1].append(ent + (bk[0],))
                            bk[0] += w
                            break
                    else:
                        banks.append([w, [ent + (0,)]])
                subbanks.extend((u, s) for u, s in banks)

                for g in range(0, len(subbanks), 2):
                    pair = subbanks[g:g + 2]
                    ps = psD.tile([128, 2 * CHUNK], f32, tag="psD")
                    et = expp.tile([128, 2 * CHUNK], bf16, tag="exp")
                    for half, (used, subs) in enumerate(pair):
                        hb = half * CHUNK
                        for p, wlo, whi, npart, kind, off in subs:
                            w = (whi - wlo + 1) * 64
                            if kind == 3:
                                nc.tensor.matmul(
                                    ps[:npart, hb:hb + CHUNK],
                                    ksT[hq, p * 128:p * 128 + npart],
                                    qT_ap(hq, 8 * c * 64, CHUNK),
                                    start=True, stop=True,
                                    skip_group_check=True)
                            elif kind == 2:
                                # tail targets partitions 64-127 so its PV
                                # can use vaug's high half directly
                                nc.tensor.matmul(
                                    ps[64:128, hb + off:hb + off + w],
                                    kT_ap(hq, (2 * p + 1) * 64, 64),
                                    qT_ap(hq, wlo * 64, w),
                                    start=True, stop=True,
                                    skip_group_check=True)
                            else:
                                nc.tensor.matmul(
                                    ps[:npart, hb + off:hb + off + w],
                                    kT_ap(hq, 2 * p * 64, 128),
                                    qT_ap(hq, wlo * 64, w),
                                    start=True, stop=True,
                                    skip_group_check=True)
                    width = (CHUNK + pair[1][0]) if len(pair) == 2 else pair[0][0]
                    nc.scalar.activation(et[:, :width], ps[:, :width], Exp,
                                         scale=SM_SCALE)
                    for half, (used, subs) in enumerate(pair):
                        hb = half * CHUNK
                        for p, wlo, whi, npart, kind, off in subs:
                            w = (whi - wlo + 1) * 64
                            if kind == 0:
                                nc.vector.tensor_mul(
                                    et[:, hb + off:hb + off + 128],
                                    et[:, hb + off:hb + off + 128], m01[:])
                            elif kind == 1:
                                nc.vector.tensor_mul(
                                    et[:, hb + off + w - 64:hb + off + w],
                                    et[:, hb + off + w - 64:hb + off + w],
                                    mB[:])
                            elif kind == 3 and p == (c - 1) // 2:
                                # strided boundary block: zero the invalid
                                # prefix columns (per-core mask data)
                                if npart == 64:
                                    nc.vector.tensor_mul(
                                        et[:64, hb:hb + CHUNK],
                                        et[:64, hb:hb + CHUNK], mbL[:])
                                else:
                                    nc.vector.tensor_mul(
                                        et[:, hb:hb + CHUNK],
                                        et[:, hb:hb + CHUNK], mbH[:])
                            if kind == 3:
                                vl = vsaug[:npart, p * 130 + hv:p * 130 + hv + 65]
                                ea = et[:npart, hb:hb + CHUNK]
                            elif kind == 2:
                                vl = vaug_ap2(p, hv)
                                ea = et[64:128, hb + off:hb + off + w]
                            else:
                                vl = vaug_ap(npart, p, hv, 65)
                                ea = et[:npart, hb + off:hb + off + w]
                            pieces.append((ea, vl, (wlo - 8 * c) * 64, w))

            if prev is not None:
                emit_phase2(*prev)
            prev = (pieces_h, c)
        emit_phase2(*prev)

    return nc


def _in_maps(q, k, v):
    import ml_dtypes
    bf = ml_dtypes.bfloat16
    maps = []
    for c in range(NCORES):
        heads = [c, c + 8]
        s = (7 - c) % 8
        qT = np.ascontiguousarray(q[:, heads, :].reshape(SEQ, 128).T).astype(bf)
        kT = np.ascontiguousarray(k[:, heads, :].reshape(SEQ, 128).T).astype(bf)
        # packed strided k blocks (7 real + zero pad), transposed, and the
        # boundary masks (zero the first s*64 columns of the boundary chunk)
        ksb = np.zeros((NSB * BS, 128), np.float32)
        vsb = np.zeros((NSB, BS, 128), np.float32)
        for b in range(7):
            j = s + 8 * b
            ksb[b * BS:(b + 1) * BS] = k[j * BS:(j + 1) * BS, heads, :].reshape(BS, 128)
            vsb[b] = v[j * BS:(j + 1) * BS, heads, :].reshape(BS, 128)
        ksT = np.ascontiguousarray(ksb.T).astype(bf)
        mbH = np.ones((128, CHUNK), np.float32)
        mbH[64:, :s * 64] = 0.0
        mbL = np.ones((64, CHUNK), np.float32)
        mbL[:, :s * 64] = 0.0
        # vaug [128, 32*130]: pair a, token p -> [V_h0 | 1 | V_h1 | 1]
        vv = v[:, heads, :].reshape(32, 128, 128)   # [a, p, hd]
        vaug = np.ones((128, 32, 130), np.float32)
        vaug[:, :, 0:64] = vv.transpose(1, 0, 2)[:, :, 0:64]
        vaug[:, :, 65:129] = vv.transpose(1, 0, 2)[:, :, 64:128]
        # vsaug [128, 4*130]: pair pr: partitions 0-63 = block 2pr, 64-127 =
        # block 2pr+1
        vsp = vsb.reshape(4, 2, BS, 128).transpose(1, 2, 0, 3).reshape(128, 4, 128)
        vsaug = np.ones((128, 4, 130), np.float32)
        vsaug[:, :, 0:64] = vsp[:, :, 0:64]
        vsaug[:, :, 65:129] = vsp[:, :, 64:128]
        maps.append({"qT": qT, "kT": kT, "ksT": ksT,
                     "vaug": vaug.reshape(128, 32 * 130).astype(bf),
                     "vsaug": vsaug.reshape(128, 4 * 130).astype(bf),
                     "mbH": mbH.astype(bf), "mbL": mbL.astype(bf)})
    return maps


def kernel(q, k, v, cu_seqlens_k=None, **_):
    from concourse.bass_utils import run_bass_kernel_spmd

    q = np.asarray(q, np.float32)
    k = np.asarray(k, np.float32)
    v = np.asarray(v, np.float32)
    if "nc" not in _cache:
        _cache["nc"] = _legalize_waits(_build_program())
    res = run_bass_kernel_spmd(_cache["nc"], _in_maps(q, k, v),
                               list(range(NCORES))).results
    out = np.empty((SEQ, N_HEADS, HEAD), np.float32)
    for c in range(NCORES):
        o = res[c]["outT"]                      # [130, SEQ]
        for hh, head in ((0, c), (1, c + 8)):
            num = o[hh * 65:hh * 65 + 64, :]    # [64, SEQ]
            den = o[hh * 65 + 64, :]            # [SEQ]
            out[:, head, :] = (num / den).T
    return out
